# revision 27
# baseline (speedup 1.0000x reference)
"""Conv1d(k=1) multi-head causal attention on 8 TRN2 NeuronCores.

Math (per batch b):
    q/k/v = x @ Wq/Wk/Wv^T          (kernel-size-1 conv == matmul)
    per head h: S = (q_h k_h^T)/8,  P = softmax(causal(S)),  ctx_h = P v_h
    out = concat_h(ctx_h) @ Wout^T + b_out

Sharding: 8 cores = 2 (batch) x 4 (head groups of 4 heads, 256 channels).
Each core computes q/k/v for its 4 heads, causal attention, and a partial
out-projection over its 256 context channels.  Host sums the 4 partials
per batch and adds the bias.

v5 schedule (single fused pipeline, engines balanced; all bf16 matmuls —
fp8 was tried and rejected: random-sign dot products keep the full ~4%
per-term e4m3 quantization error in the result, blowing the 2e-2 gate):
  - attention runs per (i4, head-pair): scores^T -> exp -> ctx^T accumulate,
    with diagonal j-blocks narrowed to the causal column range.
  - causal mask applied ON THE PE: an extra matmul (maskT stationary,
    identity moving) accumulates -1e30 above the diagonal into the scores
    PSUM, so the scores->exp chain never leaves PE->ACT.
  - exp batched per head-pair (one ACT instruction over [128, 2*w] PSUM).
  - softmax normalization: denominator row from a ones-column in v; den
    copy on ACT in PARALLEL with the ctx copy on DVE frees the single ctx
    PSUM buffer in ~1.3us so the next pair's accumulation never stalls;
    partition_broadcast (GPSIMD) + reciprocal_approx_fast (DVE).
  - projection / out-projection matmul groups are interleaved as PE filler
    inside the ACT-bound attention stream; the last TWO i4 blocks reserve
    their out-projection fillers so the PE has ~7us of independent work
    covering the final normalization chain (keeps the HAM clock at 2.4GHz
    through the tail).
  - DMA priority: x(s0)+wk+wv+wq descriptors first; the 3.5MB of
    later-needed bulk (x s1..3, wo) is gated behind wv's arrival by a tiny
    GPSIMD op so it cannot crowd the rings ahead of the prologue weights.
  - output DMA per 512-col half so the tail flush starts earlier.
"""

from contextlib import ExitStack

import numpy as np

import concourse.bass as bass
import concourse.tile as tile
from concourse import bacc, mybir
from concourse import bass_utils

F32 = mybir.dt.float32

B, S, D = 2, 2048, 1024
H_PER_CORE = 4          # heads per core
DH = 64                 # head dim
C = H_PER_CORE * DH     # 256 channels per core
E = 1024                # embed (out) dim
N_CORES = 8
NEG = -1.0e30

KC = D // 128           # 8 contraction chunks for projections
SB5 = S // 512          # 4 blocks of 512 along s
SB1 = S // 128          # 16 blocks of 128 along s


MM_DTYPE = mybir.dt.bfloat16


def build(mm_dtype=None):
    if mm_dtype is None:
        mm_dtype = MM_DTYPE
    nc = bacc.Bacc("TRN2", target_bir_lowering=False, debug=False,
                   enable_asserts=False, num_devices=N_CORES)
    xT = nc.dram_tensor("xT", (128, KC, S), mm_dtype, kind="ExternalInput").ap()
    wq = nc.dram_tensor("wq", (128, KC, C), mm_dtype, kind="ExternalInput").ap()
    wk = nc.dram_tensor("wk", (128, KC, C), mm_dtype, kind="ExternalInput").ap()
    wv = nc.dram_tensor("wv", (128, KC, C), mm_dtype, kind="ExternalInput").ap()
    wo = nc.dram_tensor("wo", (128, 2, E), mm_dtype, kind="ExternalInput").ap()
    maskT = nc.dram_tensor("maskT", (128, 128), mm_dtype, kind="ExternalInput").ap()
    ident = nc.dram_tensor("ident", (128, 128), mm_dtype, kind="ExternalInput").ap()
    o = nc.dram_tensor("o", (S, E), mm_dtype, kind="ExternalOutput").ap()

    with tile.TileContext(nc) as tc, ExitStack() as ctx:
        const = ctx.enter_context(tc.tile_pool(name="const", bufs=1))
        persist = ctx.enter_context(tc.tile_pool(name="persist", bufs=1))
        work = ctx.enter_context(tc.tile_pool(name="work", bufs=1))
        psum = ctx.enter_context(tc.tile_pool(name="psum", bufs=1, space="PSUM"))

        # ---- input DMAs: critical-path tensors first (x s0, wk, wv, wq);
        # the later-needed bulk (x s1..3, wo) is gated behind wv below ----
        xr = const.tile([128, KC, S], mm_dtype)   # [d%128, d//128, s]
        for kk in range(4):
            nc.sync.dma_start(xr[:, 2 * kk:2 * kk + 2, 0:512],
                              xT[:, 2 * kk:2 * kk + 2, 0:512])

        w_r = {}
        for name, t in (("wk", wk), ("wv", wv), ("wq", wq)):
            w_r[name] = const.tile([128, KC, C], mm_dtype, name=f"w_{name}")
        # two chunks per weight, in completion-priority order (wk fully
        # first, then wv, then wq); each dma_start gets its own ring
        for name, t, eng in (("wk", wk, nc.scalar), ("wk", wk, nc.scalar),
                             ("wv", wv, nc.scalar), ("wv", wv, nc.scalar),
                             ("wq", wq, nc.sync), ("wq", wq, nc.sync)):
            pass
        nc.scalar.dma_start(w_r["wk"][:, 0:4, :], wk[:, 0:4, :])
        nc.scalar.dma_start(w_r["wk"][:, 4:8, :], wk[:, 4:8, :])
        nc.scalar.dma_start(w_r["wv"][:, 0:4, :], wv[:, 0:4, :])
        nc.sync.dma_start(w_r["wq"][:, 0:4, :], wq[:, 0:4, :])
        nc.sync.dma_start(w_r["wq"][:, 4:8, :], wq[:, 4:8, :])
        mask_sb = const.tile([128, 128], mm_dtype)
        nc.gpsimd.dma_start(mask_sb[:], maskT)
        ident_sb = const.tile([128, 128], mm_dtype)
        nc.gpsimd.dma_start(ident_sb[:], ident)
        # second wv chunk rides the gpsimd queue to spread ring load
        nc.gpsimd.dma_start(w_r["wv"][:, 4:8, :], wv[:, 4:8, :])
        # gate: this GPSIMD op reads the wv tile, so the bulk DMAs issued
        # after it on the gpsimd queue cannot enter the rings until wv has
        # fully arrived (keeps the prologue's critical tensors first)
        dma_gate = const.tile([2, 8], mm_dtype)
        nc.gpsimd.partition_broadcast(dma_gate[:], w_r["wv"][0:1, 0, 0:8],
                                      channels=2)
        # bulk x loaded k-major (full-s rows -> 3KB contiguous packets)
        for kk in range(4):
            nc.gpsimd.dma_start(xr[:, 2 * kk:2 * kk + 2, 512:S],
                                xT[:, 2 * kk:2 * kk + 2, 512:S])
        wo_r = const.tile([128, C // 128, E], mm_dtype)
        nc.gpsimd.dma_start(wo_r[:, 0, :], wo[:, 0, :])
        nc.gpsimd.dma_start(wo_r[:, 1, :], wo[:, 1, :])

        qT = persist.tile([128, 2, S], mm_dtype)      # [c%128, c//128, s]
        kT = persist.tile([128, 2, S], mm_dtype)
        # v augmented with a ones column at free index DH -> ctx PSUM
        # partition 64 carries the softmax denominator row.
        v_aug = persist.tile([128, SB1, H_PER_CORE, DH + 1], mm_dtype)
        ctxT = persist.tile([128, 2, S], mm_dtype)
        ones_col = const.tile([128, SB1 * H_PER_CORE], F32)
        nc.vector.memset(ones_col[:], 1.0)
        nc.vector.tensor_copy(
            v_aug[:, :, :, DH],
            ones_col[:].rearrange("p (a b) -> p a b", a=SB1))

        # ---- PE warm-up: the HAM clock gate needs ~3.4us of sustained
        # activity to lift the 1.2GHz cold throttle.  While the first
        # input DMAs are in flight the PE has nothing to do, so feed it
        # dummy matmuls on a memset scratch tile (never read back) —
        # the prologue then runs at 2.4GHz from its first instruction ----
        warm = const.tile([128, 512], mm_dtype)
        nc.vector.memset(warm[:], 0.0)
        for _ in range(9):
            wps = psum.tile([128, 512], F32, tag="fill", bufs=2, name="warmps")
            nc.tensor.matmul(wps[:], warm[:, 0:128], warm[:],
                             start=True, stop=True, skip_group_check=True)

        # ---- projection emitters (used for prologue + filler) ----
        def proj_qk(w_name, dst, c2, s4):
            """One 512-col group (a full s4 block of one c2 chunk)."""
            c0 = s4 * 512
            ps = psum.tile([128, 512], F32, tag="fill", bufs=2,
                           name=f"p_{w_name}{c2}{s4}")
            for k in range(KC):
                nc.tensor.matmul(
                    ps[:],
                    w_r[w_name][:, k, c2 * 128:(c2 + 1) * 128],
                    xr[:, k, c0:c0 + 512],
                    start=(k == 0), stop=(k == KC - 1),
                    skip_group_check=True)
            nc.vector.tensor_copy(dst[:, c2, c0:c0 + 512], ps[:])

        def proj_v(s1):
            ps = psum.tile([128, C], F32, tag="fill", bufs=2, name=f"p_v{s1}")
            for k in range(KC):
                nc.tensor.matmul(
                    ps[:],
                    xr[:, k, s1 * 128:(s1 + 1) * 128],
                    w_r["wv"][:, k, :],
                    start=(k == 0), stop=(k == KC - 1),
                    skip_group_check=True)
            nc.vector.tensor_copy(
                v_aug[:, s1, :, 0:DH],
                ps[:].rearrange("p (h d) -> p h d", h=H_PER_CORE))

        out_sbs = {}

        def out_proj_half(s1, e2, act_cast=False):
            if e2 == 0:
                out_sbs[s1] = work.tile([128, E], mm_dtype, tag="osb",
                                        bufs=2, name=f"os{s1}")
            out_sb = out_sbs[s1]
            po = psum.tile([128, 512], F32, tag="fill", bufs=2,
                           name=f"po{s1}e{e2}")
            for c2 in range(2):
                nc.tensor.matmul(
                    po[:],
                    ctxT[:, c2, s1 * 128:(s1 + 1) * 128],
                    wo_r[:, c2, e2 * 512:(e2 + 1) * 512],
                    start=(c2 == 0), stop=(c2 == 1),
                    skip_group_check=True)
            if act_cast:
                nc.scalar.copy(out_sb[:, e2 * 512:(e2 + 1) * 512], po[:])
            else:
                nc.vector.tensor_copy(out_sb[:, e2 * 512:(e2 + 1) * 512], po[:])
            nc.sync.dma_start(
                o[s1 * 128:(s1 + 1) * 128, e2 * 512:(e2 + 1) * 512],
                out_sb[:, e2 * 512:(e2 + 1) * 512])
            if e2 == 1:
                del out_sbs[s1]

        def out_proj(s1):
            out_proj_half(s1, 0, act_cast=True)
            out_proj_half(s1, 1)

        # filler queue: (deadline, closure) pairs, each emits one PE
        # group.  Deadlines are (i4, p) attention positions by which the
        # filler MUST have been emitted (its output is read there);
        # drain_due force-emits overdue entries, drain_filler pops
        # opportunistically to keep the PE busy.
        filler = []

        def drain_filler(n, reserve=0):
            for _ in range(n):
                if len(filler) > reserve:
                    filler.pop(0)[1]()

        def drain_due(pos):
            while filler and filler[0][0] <= pos:
                filler.pop(0)[1]()

        ones_f32 = const.tile([1, 64], F32)
        nc.vector.memset(ones_f32[:], 1.0)

        # ---- attention for one (i4, pair) ----
        def attend_pair(i4, p, reserve=0, flush_before_norm=False):
            """Heads {2p, 2p+1} live at partitions {0:64, 64:128} of c2=p."""
            n_j = (i4 + 1) * 4
            pc = psum.tile([DH + 1, 2, 512], F32, tag="ctx", bufs=1,
                           name=f"pc{i4}p{p}")
            psts = {}
            # scores + exp, pipelined: emit sc(jt), exp(jt), then ctx(jt-1)
            for jt in range(n_j + 1):
                if jt < n_j:
                    js0 = max(jt * 128 - i4 * 512, 0)
                    sc = psum.tile([128, 2, 512], F32, tag="sc", bufs=2,
                                   name=f"sc{i4}p{p}j{jt}")
                    for hs in range(2):
                        hp = hs * 64
                        nc.tensor.matmul(
                            sc[:, hs, js0:512],
                            kT[hp:hp + 64, p, jt * 128:(jt + 1) * 128],
                            qT[hp:hp + 64, p, i4 * 512 + js0:(i4 + 1) * 512],
                            start=True, stop=True, skip_group_check=True)
                    diag = jt * 128 >= i4 * 512  # block containing the diagonal
                    if diag:
                        # accumulate -1e30 above the diagonal on the PE:
                        # sc[j, hs, g] += maskT[g, j] via identity moving op
                        # (one matmul per half: PSUM bank limits the span)
                        for hs in range(2):
                            nc.tensor.matmul(
                                sc[:, hs, js0:js0 + 128],
                                mask_sb[:], ident_sb[:],
                                start=False, stop=True,
                                skip_group_check=True)
                    pst = work.tile([128, 2, 512], mm_dtype, tag="pst", bufs=4,
                                    name=f"pt{i4}p{p}j{jt}")
                    nc.scalar.activation(
                        pst[:, :, js0:512], sc[:, :, js0:512],
                        mybir.ActivationFunctionType.Exp)
                    psts[jt] = (pst, js0)
                if jt > 0:
                    cjt = jt - 1
                    pst, js0 = psts.pop(cjt)
                    for hs in range(2):
                        nc.tensor.matmul(
                            pc[:, hs, js0:512],
                            v_aug[:, cjt, 2 * p + hs, :],
                            pst[:, hs, js0:512],
                            start=(cjt == 0), stop=(cjt == n_j - 1),
                            skip_group_check=True)
                    drain_filler(1, reserve)
            if flush_before_norm:
                drain_filler(len(filler))
                keep_warm(8)
            # normalize.  den copy on ACT runs in parallel with the ctx
            # copy on DVE, so the single ctx PSUM buffer frees in ~1.3us
            # and the next pair's first accumulation never stalls; GPSIMD
            # broadcasts the denominator; approximate reciprocal on DVE;
            # then ctxT = ctx * (1/den).
            den = work.tile([1, 2, 512], F32, tag="den", bufs=2,
                            name=f"den{i4}p{p}")
            nc.scalar.copy(den[:], pc[64:65, :, :])
            cu = work.tile([64, 2, 512], F32, tag="cu", bufs=2,
                           name=f"cu{i4}p{p}")
            nc.vector.tensor_copy(cu[:], pc[0:64, :, :])
            if flush_before_norm:
                # tail pair: shortest chain — reciprocal on the single den
                # row, then broadcast on the (idle) PE via ones x recip;
                # pipelined per head-half so mul(hs0) overlaps recip(hs1)
                rr = work.tile([1, 2, 512], F32, tag="rr", bufs=1,
                               name=f"rr{i4}p{p}")
                bcp = psum.tile([64, 2, 512], F32, tag="sc", bufs=2,
                                name=f"bcp{i4}p{p}")
                for hs in range(2):
                    nc.vector.reciprocal_approx_fast(
                        rr[:, hs, :], den[:, hs, :])
                    nc.tensor.matmul(
                        bcp[:, hs, :], ones_f32[:], rr[:, hs, :],
                        start=True, stop=True, skip_group_check=True)
                    hp = hs * 64
                    nc.vector.tensor_mul(
                        ctxT[hp:hp + 64, p, i4 * 512:(i4 + 1) * 512],
                        cu[:, hs, :], bcp[:, hs, :])
                return
            bb = work.tile([64, 2, 512], F32, tag="bb", bufs=2,
                           name=f"bb{i4}p{p}")
            nc.gpsimd.partition_broadcast(bb[:], den[:], channels=64)
            bc = work.tile([64, 2, 512], F32, tag="bc", bufs=2,
                           name=f"bc{i4}p{p}")
            nc.vector.reciprocal_approx_fast(bc[:], bb[:])
            for hs in range(2):
                hp = hs * 64
                nc.vector.tensor_mul(
                    ctxT[hp:hp + 64, p, i4 * 512:(i4 + 1) * 512],
                    cu[:, hs, :], bc[:, hs, :])

        def keep_warm(n):
            # dependency-free dummy matmuls: bridge PE idle windows so the
            # HAM clock gate never drops back to 1.2GHz
            for _ in range(n):
                wps = psum.tile([128, 512], F32, tag="fill", bufs=2,
                                name="warmps")
                nc.tensor.matmul(wps[:], warm[:, 0:128], warm[:],
                                 start=True, stop=True, skip_group_check=True)

        # ---- prologue: minimum projections for attention(i4=0, p=0);
        # pair 1's k/q run as filler inside the i4=0 attention stream ----
        proj_qk("wk", kT, 0, 0)
        proj_qk("wq", qT, 0, 0)
        keep_warm(10)
        # v projections go through the filler queue with deadline (0,0):
        # they are emitted right before attend(0,0), after k/q — so the
        # first scores/exp can be hoisted by the scheduler ahead of the
        # v matmuls if wv's DMA is still in flight
        for s1 in range(4):
            filler.append(((0, 0), lambda s1=s1: proj_v(s1)))
        filler.append(((0, 1), lambda: proj_qk("wk", kT, 1, 0)))
        filler.append(((0, 1), lambda: proj_qk("wq", qT, 1, 0)))

        # ---- main loop ----
        for i4 in range(SB5):
            # queue filler: projections for the next s-block, out-proj for
            # the previous (already-normalized) one
            if i4 + 1 < SB5:
                s4 = i4 + 1
                for c2 in range(2):
                    filler.append(((s4, 0), lambda c2=c2, s4=s4:
                                   proj_qk("wk", kT, c2, s4)))
                for s1 in range(4 * s4, 4 * s4 + 4):
                    filler.append(((s4, 0), lambda s1=s1: proj_v(s1)))
                for c2 in range(2):
                    filler.append(((s4, 0), lambda c2=c2, s4=s4:
                                   proj_qk("wq", qT, c2, s4)))
            if i4 > 0:
                for s1 in range(4 * (i4 - 1), 4 * i4):
                    for e2 in range(2):
                        filler.append(((SB5, 0), lambda s1=s1, e2=e2:
                                       out_proj_half(s1, e2)))
            # the last two i4 blocks RESERVE their out-proj fillers: the
            # final normalization chain (~6us of DVE/GPSIMD latency) then
            # has real PE work to cover it
            reserve = {SB5 - 2: 8, SB5 - 1: 16}.get(i4, 0)
            last = i4 == SB5 - 1
            for p in range(2):
                drain_due((i4, p))
                attend_pair(i4, p, reserve, flush_before_norm=(last and p == 1))
            if not last:
                drain_filler(2, reserve)
        while filler:
            filler.pop(0)[1]()
        for s1 in range(12, 16):
            out_proj(s1)

    nc.compile()
    return nc


def make_maskT():
    # maskT[g, j] = 0 if g >= j else NEG; accumulated into scores via
    # sc[j, g] += sum_k maskT[k, j] * I[k, g] = maskT[g, j]
    g = np.arange(128)[:, None]
    j = np.arange(128)[None, :]
    return np.where(g >= j, 0.0, NEG).astype(np.float32)


def make_ident():
    return np.eye(128, dtype=np.float32)


def _pk_layout(m):
    """[D, N] -> [128, D//128, N]: row d = k*128 + p."""
    d, n = m.shape
    return np.ascontiguousarray(
        np.asarray(m).reshape(d // 128, 128, n).transpose(1, 0, 2))


def make_in_maps(x, wq, wk, wv, w_out, mm_dtype=None):
    """Per-core inputs. Core c: batch c//4, head-group c%4."""
    if mm_dtype is None:
        mm_dtype = MM_DTYPE
    if mm_dtype == mybir.dt.bfloat16:
        import ml_dtypes
        cast = lambda a: np.ascontiguousarray(a).astype(ml_dtypes.bfloat16)
    else:
        cast = lambda a: np.ascontiguousarray(a, dtype=np.float32)
    maskT = cast(make_maskT())
    ident = cast(make_ident())
    scale = DH ** (-0.5)
    in_maps = []
    for c in range(N_CORES):
        b, hg = c // 4, c % 4
        cs = slice(hg * C, (hg + 1) * C)
        in_maps.append({
            "xT": cast(_pk_layout(x[b].T)),
            "wq": cast(_pk_layout((wq[cs, :, 0] * scale).T)),
            "wk": cast(_pk_layout(wk[cs, :, 0].T)),
            "wv": cast(_pk_layout(wv[cs, :, 0].T)),
            "wo": cast(_pk_layout(w_out[:, cs].T)),
            "maskT": maskT,
            "ident": ident,
        })
    return in_maps


_NC_CACHE = {}


def get_nc(mm_dtype=None):
    if mm_dtype is None:
        mm_dtype = MM_DTYPE
    key = str(mm_dtype)
    if key not in _NC_CACHE:
        _NC_CACHE[key] = build(mm_dtype)
    return _NC_CACHE[key]


def kernel(x, attn_mask, wq, wk, wv, w_out, b_out):
    x = np.asarray(x, dtype=np.float32)
    nc = get_nc()
    in_maps = make_in_maps(x, np.asarray(wq), np.asarray(wk),
                           np.asarray(wv), np.asarray(w_out))
    res = bass_utils.run_bass_kernel_spmd(nc, in_maps,
                                          core_ids=list(range(N_CORES)))
    out = np.zeros((B, S, E), dtype=np.float32)
    for c in range(N_CORES):
        out[c // 4] += np.asarray(res.results[c]["o"], dtype=np.float32)
    out += np.asarray(b_out, dtype=np.float32)
    return out


# revision 28
# speedup vs baseline: 1.0341x; 1.0341x over previous
"""Conv1d(k=1) multi-head causal attention on 8 TRN2 NeuronCores.

Math (per batch b):
    q/k/v = x @ Wq/Wk/Wv^T          (kernel-size-1 conv == matmul)
    per head h: S = (q_h k_h^T)/8,  P = softmax(causal(S)),  ctx_h = P v_h
    out = concat_h(ctx_h) @ Wout^T + b_out

Sharding: 8 cores = 2 (batch) x 4 (head groups of 4 heads, 256 channels).
Each core computes q/k/v for its 4 heads, causal attention, and a partial
out-projection over its 256 context channels.  Host sums the 4 partials
per batch and adds the bias.

v5 schedule (single fused pipeline, engines balanced; all bf16 matmuls —
fp8 was tried and rejected: random-sign dot products keep the full ~4%
per-term e4m3 quantization error in the result, blowing the 2e-2 gate):
  - attention runs per (i4, head-pair): scores^T -> exp -> ctx^T accumulate,
    with diagonal j-blocks narrowed to the causal column range.
  - causal mask applied ON THE PE: an extra matmul (maskT stationary,
    identity moving) accumulates -1e30 above the diagonal into the scores
    PSUM, so the scores->exp chain never leaves PE->ACT.
  - exp batched per head-pair (one ACT instruction over [128, 2*w] PSUM).
  - softmax normalization: denominator row from a ones-column in v; den
    copy on ACT in PARALLEL with the ctx copy on DVE frees the single ctx
    PSUM buffer in ~1.3us so the next pair's accumulation never stalls;
    partition_broadcast (GPSIMD) + reciprocal_approx_fast (DVE).
  - projection / out-projection matmul groups are interleaved as PE filler
    inside the ACT-bound attention stream; the last TWO i4 blocks reserve
    their out-projection fillers so the PE has ~7us of independent work
    covering the final normalization chain (keeps the HAM clock at 2.4GHz
    through the tail).
  - DMA priority: x(s0)+wk+wv+wq descriptors first; the 3.5MB of
    later-needed bulk (x s1..3, wo) is gated behind wv's arrival by a tiny
    GPSIMD op so it cannot crowd the rings ahead of the prologue weights.
  - output DMA per 512-col half so the tail flush starts earlier.
"""

from contextlib import ExitStack

import numpy as np

import concourse.bass as bass
import concourse.tile as tile
from concourse import bacc, mybir
from concourse import bass_utils

F32 = mybir.dt.float32

B, S, D = 2, 2048, 1024
H_PER_CORE = 4          # heads per core
DH = 64                 # head dim
C = H_PER_CORE * DH     # 256 channels per core
E = 1024                # embed (out) dim
N_CORES = 8
NEG = -1.0e30

KC = D // 128           # 8 contraction chunks for projections
SB5 = S // 512          # 4 blocks of 512 along s
SB1 = S // 128          # 16 blocks of 128 along s


MM_DTYPE = mybir.dt.bfloat16


def build(mm_dtype=None):
    if mm_dtype is None:
        mm_dtype = MM_DTYPE
    nc = bacc.Bacc("TRN2", target_bir_lowering=False, debug=False,
                   enable_asserts=False, num_devices=N_CORES)
    xT = nc.dram_tensor("xT", (128, KC, S), mm_dtype, kind="ExternalInput").ap()
    wq = nc.dram_tensor("wq", (128, KC, C), mm_dtype, kind="ExternalInput").ap()
    wk = nc.dram_tensor("wk", (128, KC, C), mm_dtype, kind="ExternalInput").ap()
    wv = nc.dram_tensor("wv", (128, KC, C), mm_dtype, kind="ExternalInput").ap()
    wo = nc.dram_tensor("wo", (128, 2, E), mm_dtype, kind="ExternalInput").ap()
    maskT = nc.dram_tensor("maskT", (128, 128), mm_dtype, kind="ExternalInput").ap()
    ident = nc.dram_tensor("ident", (128, 128), mm_dtype, kind="ExternalInput").ap()
    o = nc.dram_tensor("o", (S, E), mm_dtype, kind="ExternalOutput").ap()

    with tile.TileContext(nc) as tc, ExitStack() as ctx:
        const = ctx.enter_context(tc.tile_pool(name="const", bufs=1))
        persist = ctx.enter_context(tc.tile_pool(name="persist", bufs=1))
        work = ctx.enter_context(tc.tile_pool(name="work", bufs=1))
        psum = ctx.enter_context(tc.tile_pool(name="psum", bufs=1, space="PSUM"))

        # ---- input DMAs: critical-path tensors first (x s0, wk, wv, wq);
        # the later-needed bulk (x s1..3, wo) is gated behind wv below ----
        xr = const.tile([128, KC, S], mm_dtype)   # [d%128, d//128, s]
        for kk in range(4):
            nc.sync.dma_start(xr[:, 2 * kk:2 * kk + 2, 0:512],
                              xT[:, 2 * kk:2 * kk + 2, 0:512])

        w_r = {}
        for name, t in (("wk", wk), ("wv", wv), ("wq", wq)):
            w_r[name] = const.tile([128, KC, C], mm_dtype, name=f"w_{name}")
        # two chunks per weight, in completion-priority order (wk fully
        # first, then wv, then wq); each dma_start gets its own ring
        for name, t, eng in (("wk", wk, nc.scalar), ("wk", wk, nc.scalar),
                             ("wv", wv, nc.scalar), ("wv", wv, nc.scalar),
                             ("wq", wq, nc.sync), ("wq", wq, nc.sync)):
            pass
        nc.scalar.dma_start(w_r["wk"][:, 0:4, :], wk[:, 0:4, :])
        nc.scalar.dma_start(w_r["wk"][:, 4:8, :], wk[:, 4:8, :])
        nc.scalar.dma_start(w_r["wv"][:, 0:4, :], wv[:, 0:4, :])
        nc.sync.dma_start(w_r["wq"][:, 0:4, :], wq[:, 0:4, :])
        nc.sync.dma_start(w_r["wq"][:, 4:8, :], wq[:, 4:8, :])
        mask_sb = const.tile([128, 128], mm_dtype)
        nc.gpsimd.dma_start(mask_sb[:], maskT)
        ident_sb = const.tile([128, 128], mm_dtype)
        nc.gpsimd.dma_start(ident_sb[:], ident)
        # second wv chunk rides the gpsimd queue to spread ring load
        nc.gpsimd.dma_start(w_r["wv"][:, 4:8, :], wv[:, 4:8, :])
        # gate: this GPSIMD op reads the wv tile, so the bulk DMAs issued
        # after it on the gpsimd queue cannot enter the rings until wv has
        # fully arrived (keeps the prologue's critical tensors first)
        dma_gate = const.tile([2, 8], mm_dtype)
        nc.gpsimd.partition_broadcast(dma_gate[:], w_r["wv"][0:1, 0, 0:8],
                                      channels=2)
        # bulk x loaded k-major (full-s rows -> 3KB contiguous packets)
        for kk in range(4):
            nc.gpsimd.dma_start(xr[:, 2 * kk:2 * kk + 2, 512:S],
                                xT[:, 2 * kk:2 * kk + 2, 512:S])
        wo_r = const.tile([128, C // 128, E], mm_dtype)
        nc.gpsimd.dma_start(wo_r[:, 0, :], wo[:, 0, :])
        nc.gpsimd.dma_start(wo_r[:, 1, :], wo[:, 1, :])

        qT = persist.tile([128, 2, S], mm_dtype)      # [c%128, c//128, s]
        kT = persist.tile([128, 2, S], mm_dtype)
        # v augmented with a ones column at free index DH -> ctx PSUM
        # partition 64 carries the softmax denominator row.
        v_aug = persist.tile([128, SB1, H_PER_CORE, DH + 1], mm_dtype)
        ctxT = persist.tile([128, 2, S], mm_dtype)
        ones_col = const.tile([128, SB1 * H_PER_CORE], F32)
        nc.vector.memset(ones_col[:], 1.0)
        nc.vector.tensor_copy(
            v_aug[:, :, :, DH],
            ones_col[:].rearrange("p (a b) -> p a b", a=SB1))

        # ---- PE warm-up: the HAM clock gate needs ~3.4us of sustained
        # activity to lift the 1.2GHz cold throttle.  While the first
        # input DMAs are in flight the PE has nothing to do, so feed it
        # dummy matmuls on a memset scratch tile (never read back) —
        # the prologue then runs at 2.4GHz from its first instruction ----
        warm = const.tile([128, 512], mm_dtype)
        nc.vector.memset(warm[:], 0.0)
        for _ in range(9):
            wps = psum.tile([128, 512], F32, tag="fill", bufs=2, name="warmps")
            nc.tensor.matmul(wps[:], warm[:, 0:128], warm[:],
                             start=True, stop=True, skip_group_check=True)

        # ---- projection emitters (used for prologue + filler) ----
        def proj_qk(w_name, dst, c2, s4):
            """One 512-col group (a full s4 block of one c2 chunk)."""
            c0 = s4 * 512
            ps = psum.tile([128, 512], F32, tag="fill", bufs=2,
                           name=f"p_{w_name}{c2}{s4}")
            for k in range(KC):
                nc.tensor.matmul(
                    ps[:],
                    w_r[w_name][:, k, c2 * 128:(c2 + 1) * 128],
                    xr[:, k, c0:c0 + 512],
                    start=(k == 0), stop=(k == KC - 1),
                    skip_group_check=True)
            nc.vector.tensor_copy(dst[:, c2, c0:c0 + 512], ps[:])

        def proj_v(s1):
            ps = psum.tile([128, C], F32, tag="fill", bufs=2, name=f"p_v{s1}")
            for k in range(KC):
                nc.tensor.matmul(
                    ps[:],
                    xr[:, k, s1 * 128:(s1 + 1) * 128],
                    w_r["wv"][:, k, :],
                    start=(k == 0), stop=(k == KC - 1),
                    skip_group_check=True)
            nc.vector.tensor_copy(
                v_aug[:, s1, :, 0:DH],
                ps[:].rearrange("p (h d) -> p h d", h=H_PER_CORE))

        out_sbs = {}

        def out_proj_half(s1, e2, act_cast=False):
            if e2 == 0:
                out_sbs[s1] = work.tile([128, E], mm_dtype, tag="osb",
                                        bufs=2, name=f"os{s1}")
            out_sb = out_sbs[s1]
            po = psum.tile([128, 512], F32, tag="fill", bufs=2,
                           name=f"po{s1}e{e2}")
            for c2 in range(2):
                nc.tensor.matmul(
                    po[:],
                    ctxT[:, c2, s1 * 128:(s1 + 1) * 128],
                    wo_r[:, c2, e2 * 512:(e2 + 1) * 512],
                    start=(c2 == 0), stop=(c2 == 1),
                    skip_group_check=True)
            if act_cast:
                nc.scalar.copy(out_sb[:, e2 * 512:(e2 + 1) * 512], po[:])
            else:
                nc.vector.tensor_copy(out_sb[:, e2 * 512:(e2 + 1) * 512], po[:])
            nc.sync.dma_start(
                o[s1 * 128:(s1 + 1) * 128, e2 * 512:(e2 + 1) * 512],
                out_sb[:, e2 * 512:(e2 + 1) * 512])
            if e2 == 1:
                del out_sbs[s1]

        def out_proj(s1):
            out_proj_half(s1, 0, act_cast=True)
            out_proj_half(s1, 1)

        # filler queue: (deadline, closure) pairs, each emits one PE
        # group.  Deadlines are (i4, p) attention positions by which the
        # filler MUST have been emitted (its output is read there);
        # drain_due force-emits overdue entries, drain_filler pops
        # opportunistically to keep the PE busy.
        filler = []

        def drain_filler(n, reserve=0):
            for _ in range(n):
                if len(filler) > reserve:
                    filler.pop(0)[1]()

        def drain_due(pos):
            while filler and filler[0][0] <= pos:
                filler.pop(0)[1]()

        ones_f32 = const.tile([1, 64], F32)
        nc.vector.memset(ones_f32[:], 1.0)

        # ---- attention for one (i4, pair) ----
        def attend_pair(i4, p, reserve=0, flush_before_norm=False):
            """Heads {2p, 2p+1} live at partitions {0:64, 64:128} of c2=p."""
            n_j = (i4 + 1) * 4
            pc = psum.tile([DH + 1, 2, 512], F32, tag="ctx", bufs=1,
                           name=f"pc{i4}p{p}")
            psts = {}
            # scores + exp, pipelined: emit sc(jt), exp(jt), then ctx(jt-1)
            for jt in range(n_j + 1):
                if jt < n_j:
                    js0 = max(jt * 128 - i4 * 512, 0)
                    sc = psum.tile([128, 2, 512], F32, tag="sc", bufs=2,
                                   name=f"sc{i4}p{p}j{jt}")
                    for hs in range(2):
                        hp = hs * 64
                        nc.tensor.matmul(
                            sc[:, hs, js0:512],
                            kT[hp:hp + 64, p, jt * 128:(jt + 1) * 128],
                            qT[hp:hp + 64, p, i4 * 512 + js0:(i4 + 1) * 512],
                            start=True, stop=True, skip_group_check=True)
                    diag = jt * 128 >= i4 * 512  # block containing the diagonal
                    if diag:
                        # accumulate -1e30 above the diagonal on the PE:
                        # sc[j, hs, g] += maskT[g, j] via identity moving op
                        # (one matmul per half: PSUM bank limits the span)
                        for hs in range(2):
                            nc.tensor.matmul(
                                sc[:, hs, js0:js0 + 128],
                                mask_sb[:], ident_sb[:],
                                start=False, stop=True,
                                skip_group_check=True)
                    pst = work.tile([128, 2, 512], mm_dtype, tag="pst", bufs=4,
                                    name=f"pt{i4}p{p}j{jt}")
                    nc.scalar.activation(
                        pst[:, :, js0:512], sc[:, :, js0:512],
                        mybir.ActivationFunctionType.Exp)
                    psts[jt] = (pst, js0)
                if jt > 0:
                    cjt = jt - 1
                    pst, js0 = psts.pop(cjt)
                    for hs in range(2):
                        nc.tensor.matmul(
                            pc[:, hs, js0:512],
                            v_aug[:, cjt, 2 * p + hs, :],
                            pst[:, hs, js0:512],
                            start=(cjt == 0), stop=(cjt == n_j - 1),
                            skip_group_check=True)
                    drain_filler(1, reserve)
            if flush_before_norm:
                drain_filler(len(filler))
            # normalize.  den copy on ACT runs in parallel with the ctx
            # copy on DVE, so the single ctx PSUM buffer frees in ~1.3us
            # and the next pair's first accumulation never stalls; GPSIMD
            # broadcasts the denominator; approximate reciprocal on DVE;
            # then ctxT = ctx * (1/den).
            den = work.tile([1, 2, 512], F32, tag="den", bufs=2,
                            name=f"den{i4}p{p}")
            nc.scalar.copy(den[:], pc[64:65, :, :])
            cu = work.tile([64, 2, 512], F32, tag="cu", bufs=2,
                           name=f"cu{i4}p{p}")
            nc.vector.tensor_copy(cu[:], pc[0:64, :, :])
            if flush_before_norm:
                # tail pair: shortest chain — reciprocal on the single den
                # row, then broadcast on the (idle) PE via ones x recip;
                # pipelined per head-half so mul(hs0) overlaps recip(hs1)
                rr = work.tile([1, 2, 512], F32, tag="rr", bufs=1,
                               name=f"rr{i4}p{p}")
                bcp = psum.tile([64, 2, 512], F32, tag="sc", bufs=2,
                                name=f"bcp{i4}p{p}")
                for hs in range(2):
                    nc.vector.reciprocal_approx_fast(
                        rr[:, hs, :], den[:, hs, :])
                    nc.tensor.matmul(
                        bcp[:, hs, :], ones_f32[:], rr[:, hs, :],
                        start=True, stop=True, skip_group_check=True)
                    hp = hs * 64
                    nc.vector.tensor_mul(
                        ctxT[hp:hp + 64, p, i4 * 512:(i4 + 1) * 512],
                        cu[:, hs, :], bcp[:, hs, :])
                return
            bb = work.tile([64, 2, 512], F32, tag="bb", bufs=2,
                           name=f"bb{i4}p{p}")
            nc.gpsimd.partition_broadcast(bb[:], den[:], channels=64)
            bc = work.tile([64, 2, 512], F32, tag="bc", bufs=2,
                           name=f"bc{i4}p{p}")
            nc.vector.reciprocal_approx_fast(bc[:], bb[:])
            for hs in range(2):
                hp = hs * 64
                nc.vector.tensor_mul(
                    ctxT[hp:hp + 64, p, i4 * 512:(i4 + 1) * 512],
                    cu[:, hs, :], bc[:, hs, :])

        # ---- prologue: minimum projections for attention(i4=0, p=0);
        # pair 1's k/q run as filler inside the i4=0 attention stream ----
        proj_qk("wk", kT, 0, 0)
        proj_qk("wq", qT, 0, 0)
        # v projections go through the filler queue with deadline (0,0):
        # they are emitted right before attend(0,0), after k/q — so the
        # first scores/exp can be hoisted by the scheduler ahead of the
        # v matmuls if wv's DMA is still in flight
        for s1 in range(4):
            filler.append(((0, 0), lambda s1=s1: proj_v(s1)))
        filler.append(((0, 1), lambda: proj_qk("wk", kT, 1, 0)))
        filler.append(((0, 1), lambda: proj_qk("wq", qT, 1, 0)))

        # ---- main loop ----
        for i4 in range(SB5):
            # queue filler: projections for the next s-block, out-proj for
            # the previous (already-normalized) one
            if i4 + 1 < SB5:
                s4 = i4 + 1
                for c2 in range(2):
                    filler.append(((s4, 0), lambda c2=c2, s4=s4:
                                   proj_qk("wk", kT, c2, s4)))
                for s1 in range(4 * s4, 4 * s4 + 4):
                    filler.append(((s4, 0), lambda s1=s1: proj_v(s1)))
                for c2 in range(2):
                    filler.append(((s4, 0), lambda c2=c2, s4=s4:
                                   proj_qk("wq", qT, c2, s4)))
            if i4 > 0:
                for s1 in range(4 * (i4 - 1), 4 * i4):
                    for e2 in range(2):
                        filler.append(((SB5, 0), lambda s1=s1, e2=e2:
                                       out_proj_half(s1, e2)))
            # the last two i4 blocks RESERVE their out-proj fillers: the
            # final normalization chain (~6us of DVE/GPSIMD latency) then
            # has real PE work to cover it
            reserve = {SB5 - 2: 8, SB5 - 1: 16}.get(i4, 0)
            last = i4 == SB5 - 1
            for p in range(2):
                drain_due((i4, p))
                attend_pair(i4, p, reserve, flush_before_norm=(last and p == 1))
            if not last:
                drain_filler(2, reserve)
        while filler:
            filler.pop(0)[1]()
        for s1 in range(12, 16):
            out_proj(s1)

    nc.compile()
    return nc


def make_maskT():
    # maskT[g, j] = 0 if g >= j else NEG; accumulated into scores via
    # sc[j, g] += sum_k maskT[k, j] * I[k, g] = maskT[g, j]
    g = np.arange(128)[:, None]
    j = np.arange(128)[None, :]
    return np.where(g >= j, 0.0, NEG).astype(np.float32)


def make_ident():
    return np.eye(128, dtype=np.float32)


def _pk_layout(m):
    """[D, N] -> [128, D//128, N]: row d = k*128 + p."""
    d, n = m.shape
    return np.ascontiguousarray(
        np.asarray(m).reshape(d // 128, 128, n).transpose(1, 0, 2))


def make_in_maps(x, wq, wk, wv, w_out, mm_dtype=None):
    """Per-core inputs. Core c: batch c//4, head-group c%4."""
    if mm_dtype is None:
        mm_dtype = MM_DTYPE
    if mm_dtype == mybir.dt.bfloat16:
        import ml_dtypes
        cast = lambda a: np.ascontiguousarray(a).astype(ml_dtypes.bfloat16)
    else:
        cast = lambda a: np.ascontiguousarray(a, dtype=np.float32)
    maskT = cast(make_maskT())
    ident = cast(make_ident())
    scale = DH ** (-0.5)
    in_maps = []
    for c in range(N_CORES):
        b, hg = c // 4, c % 4
        cs = slice(hg * C, (hg + 1) * C)
        in_maps.append({
            "xT": cast(_pk_layout(x[b].T)),
            "wq": cast(_pk_layout((wq[cs, :, 0] * scale).T)),
            "wk": cast(_pk_layout(wk[cs, :, 0].T)),
            "wv": cast(_pk_layout(wv[cs, :, 0].T)),
            "wo": cast(_pk_layout(w_out[:, cs].T)),
            "maskT": maskT,
            "ident": ident,
        })
    return in_maps


_NC_CACHE = {}


def get_nc(mm_dtype=None):
    if mm_dtype is None:
        mm_dtype = MM_DTYPE
    key = str(mm_dtype)
    if key not in _NC_CACHE:
        _NC_CACHE[key] = build(mm_dtype)
    return _NC_CACHE[key]


def kernel(x, attn_mask, wq, wk, wv, w_out, b_out):
    x = np.asarray(x, dtype=np.float32)
    nc = get_nc()
    in_maps = make_in_maps(x, np.asarray(wq), np.asarray(wk),
                           np.asarray(wv), np.asarray(w_out))
    res = bass_utils.run_bass_kernel_spmd(nc, in_maps,
                                          core_ids=list(range(N_CORES)))
    out = np.zeros((B, S, E), dtype=np.float32)
    for c in range(N_CORES):
        out[c // 4] += np.asarray(res.results[c]["o"], dtype=np.float32)
    out += np.asarray(b_out, dtype=np.float32)
    return out
